# revision 27
# baseline (speedup 1.0000x reference)
"""AGRNN (gnn_message_passing) Trainium2 kernel, 8 NeuronCores SPMD.

Strategy (v2):
  - Edge MLPs decomposed through the concat: relu([f_src|f_dst|s_f] @ W_e)
    = relu(A[src] + B[dst] + [s_f|1] @ [We3;b_e]) with A = feat @ We1,
    B = feat @ We2.
  - Nodes are RELABELED on the host so that each 128-dst block has exactly
    1024 incident edges (greedy balance + swap repair). No padding anywhere,
    and dst-blocks of B/BL stay SBUF-resident.
  - A and AL (src-side projections) are stored fp8-e4m3, all-gathered in two
    column halves for collective/compute overlap, and gathered per edge.
    Messages m/ml are fp8; mean-normalization (1/cnt) is folded into the
    relu via the activation engine's per-partition scale.
  - Aggregation: one-hot matmul with lhsT=oh (fp8 DoubleRow: 256 edges per
    matmul), producing agg[dst, D] which is DMA-transposed (xbar) into the
    [D, node] layout the node-update consumes.
  - Predictor projections computed transposed ([C, node]) with N=512
    matmuls, DMA-transposed back, all-gathered as bf16, and the readout is
    per-edge row gathers + adds.
"""

import os
import numpy as np
import ml_dtypes

import concourse.bass as bass
import concourse.mybir as mybir
import concourse.tile as tile
from concourse import bacc, library_config
from concourse.bass_utils import run_bass_kernel_spmd

BF = ml_dtypes.bfloat16
F8 = ml_dtypes.float8_e4m3
F32 = np.float32

N, D, DL, DS, C = 16384, 1024, 300, 16, 117
DLP = 384              # language dim padded for matmul K-tiles
ALP = 512              # fp8 AL row bytes (gather rows must be 256B multiple)
CP = 128               # class dim padded
E, E_RO = 131072, 32768
CORES = 8
NS = N // CORES        # 2048 nodes / core
NGRP = NS // 128       # 16 dst groups / core
GS = 8                 # 1024 edges per group exactly (balanced)
GMAX = GS * 128
RO_LOC = E_RO // CORES # 4096 readout edges / core
ROS = RO_LOC // 128

DOUBLE_ROW = os.environ.get("KDR", "1") == "1"

dt = mybir.dt


# --------------------------------------------------------------------------
# host-side preprocessing
# --------------------------------------------------------------------------

def _gather_idx_layout(idx, width):
    """dma_gather index layout: idx j at [j % 16, j // 16], 16-row block
    replicated across the 8 GPSIMD Q7 cores (128 partitions)."""
    n = len(idx)
    assert n % 16 == 0
    blk = np.zeros((16, width), np.int16)
    blk[:, : n // 16] = np.asarray(idx, np.int16).reshape(n // 16, 16).T
    return np.tile(blk, (8, 1))


def _balance_groups(cnt):
    """Assign each node to one of N/128 groups of exactly 128 nodes such
    that each group's total edge count is <= E/(N/128) (=1024)."""
    import heapq
    G = N // 128
    cap = E // G
    order = np.argsort(-cnt, kind="stable")
    assign = np.zeros(N, np.int64)
    loads = np.zeros(G, np.int64)
    fill = np.zeros(G, np.int64)
    heap = [(0, g) for g in range(G)]
    heapq.heapify(heap)
    for n in order:
        spill = []
        while True:
            l, g = heapq.heappop(heap)
            if fill[g] < 128:
                break
            spill.append((l, g))
        assign[n] = g
        fill[g] += 1
        loads[g] += cnt[n]
        heapq.heappush(heap, (loads[g], g))
        for it in spill:
            heapq.heappush(heap, it)
    # repair: move small nodes from overloaded to underloaded groups by
    # swapping a cnt-k node with a cnt-(k-1) node.
    nodes_by = {}
    for n in range(N):
        nodes_by.setdefault((assign[n], cnt[n]), []).append(n)
    guard = 0
    while loads.max() > cap and guard < 10000:
        guard += 1
        g_hi = int(np.argmax(loads))
        g_lo = int(np.argmin(loads))
        done = False
        for k in range(1, int(cnt.max()) + 1):
            hi_list = nodes_by.get((g_hi, k))
            lo_list = nodes_by.get((g_lo, k - 1))
            if hi_list and lo_list is not None and len(lo_list) > 0:
                a = hi_list.pop()
                b = lo_list.pop()
                assign[a], assign[b] = g_lo, g_hi
                nodes_by.setdefault((g_lo, k), []).append(a)
                nodes_by.setdefault((g_hi, k - 1), []).append(b)
                loads[g_hi] += (k - 1) - k
                loads[g_lo] += k - (k - 1)
                done = True
                break
        if not done:
            break
    assert loads.max() <= cap, f"group balance failed: max={loads.max()}"
    # positions within each group
    new2old = np.zeros(N, np.int64)
    fill2 = np.zeros(G, np.int64)
    for n in range(N):
        g = assign[n]
        new2old[g * 128 + fill2[g]] = n
        fill2[g] += 1
    old2new = np.zeros(N, np.int64)
    old2new[new2old] = np.arange(N)
    return new2old, old2new


def _preprocess(inputs):
    feat = np.asarray(inputs["feat"], F32)
    w2v = np.asarray(inputs["word2vec"], F32)
    s_f = np.asarray(inputs["s_f"], F32)
    s_f_ro = np.asarray(inputs["s_f_ro"], F32)
    W_e, b_e = np.asarray(inputs["W_e"], F32), np.asarray(inputs["b_e"], F32)
    W_el, b_el = np.asarray(inputs["W_el"], F32), np.asarray(inputs["b_el"], F32)
    W_nu, b_nu = np.asarray(inputs["W_nu"], F32), np.asarray(inputs["b_nu"], F32)
    W_nul, b_nul = np.asarray(inputs["W_nul"], F32), np.asarray(inputs["b_nul"], F32)
    W_p, b_p = np.asarray(inputs["W_p"], F32), np.asarray(inputs["b_p"], F32)
    es = np.asarray(inputs["edge_src"], np.int64)
    ed = np.asarray(inputs["edge_dst"], np.int64)
    ro_s = np.asarray(inputs["ro_src"], np.int64)
    ro_d = np.asarray(inputs["ro_dst"], np.int64)

    cnt = np.bincount(ed, minlength=N)
    inv_cnt = (1.0 / np.maximum(cnt, 1.0)).astype(F32)

    new2old, old2new = _balance_groups(cnt)
    feat_n = feat[new2old]
    w2v_n = w2v[new2old]
    es_n = old2new[es]
    ed_n = old2new[ed]
    inv_n = inv_cnt[new2old]          # indexed by new id

    # dst-sort edges under the new labeling; each global group g has
    # exactly <= 1024 edges.
    perm = np.argsort(ed_n, kind="stable")
    es_s, ed_s = es_n[perm], ed_n[perm]
    sf_s = s_f[perm]

    G = N // 128
    grp_of = ed_s // 128
    counts = np.bincount(grp_of, minlength=G)
    assert counts.max() <= GMAX

    src_pad = np.zeros((CORES, NGRP, GMAX), np.int64)
    ohv_pad = np.zeros((CORES, NGRP, GMAX), F32)
    dslot_pad = np.zeros((CORES, NGRP, GMAX), np.int64)
    real_pad = np.zeros((CORES, NGRP, GMAX), bool)
    sf_pad = np.zeros((CORES, NGRP, GMAX, DS), F32)
    starts = np.concatenate([[0], np.cumsum(counts)])
    for gg in range(G):
        c, g = divmod(gg, NGRP)
        lo, hi = starts[gg], starts[gg + 1]
        n = hi - lo
        src_pad[c, g, :n] = es_s[lo:hi]
        dslot_pad[c, g, :n] = ed_s[lo:hi] - gg * 128
        ohv_pad[c, g, :n] = inv_n[ed_s[lo:hi]]
        real_pad[c, g, :n] = True
        sf_pad[c, g, :n] = sf_s[lo:hi]

    # one-hot [C, NGRP, 128(edge-in-chunk), GS, 128(dst)] fp8 0/1
    oh = np.zeros((CORES, NGRP, GMAX, 128), F32)
    ohT = np.zeros((CORES, NGRP, 128, GMAX), F32)
    ar = np.arange(GMAX)
    for c in range(CORES):
        for g in range(NGRP):
            d = dslot_pad[c, g]
            r = real_pad[c, g]
            oh[c, g, ar[r], d[r]] = 1.0
            ohT[c, g, d[r], ar[r]] = 1.0
    # edge-part layout [128, GS*128]
    oh_dev = np.ascontiguousarray(
        oh.reshape(CORES, NGRP, GS, 128, 128).transpose(0, 1, 3, 2, 4)
    ).reshape(CORES, NGRP, 128, GS * 128)

    # per-edge inv_cnt in [128, NGRP*GS] layout (partition = edge%128)
    invc = np.zeros((CORES, 128, NGRP * GS), F32)
    for c in range(CORES):
        v = ohv_pad[c].reshape(NGRP, GS, 128)      # [g, s, p]
        invc[c] = v.transpose(2, 0, 1).reshape(128, NGRP * GS)

    # sfT with ones row (folds b_e via We3b)
    sfT = np.zeros((CORES, NGRP, DS + 1, GMAX), F32)
    sfT[:, :, :DS, :] = sf_pad.transpose(0, 1, 3, 2)
    sfT[:, :, DS, :] = 1.0

    # weight splits
    We1, We2, We3 = W_e[:D], W_e[D : 2 * D], W_e[2 * D :]
    We3b = np.concatenate([We3, b_e[None, :]], 0)          # [17, D]
    Wel1, Wel2 = W_el[:DL], W_el[DL:]
    Wnu1, Wnu2 = W_nu[:D], W_nu[D:]
    Wnul1, Wnul2 = W_nul[:DL], W_nul[DL:]
    Wp1, Wp2 = W_p[:D], W_p[D : D + DL]
    Wp3 = W_p[D + DL : D + DL + DS]
    Wp4 = W_p[D + DL + DS : D + 2 * DL + DS]
    Wp5 = W_p[D + 2 * DL + DS :]

    def padc(w, rows, cols):
        out = np.zeros((rows, cols), F32)
        out[: w.shape[0], : w.shape[1]] = w
        return out

    Wel1p = padc(Wel1, DLP, DLP)
    Wel2p = padc(Wel2, DLP, DLP)
    Wnul1p = padc(Wnul1, DLP, DLP)
    Wnul2p = padc(Wnul2, DLP, DLP)
    belp = np.zeros((DLP,), F32)
    belp[:DL] = b_el
    Wp3b = padc(np.concatenate([Wp3, b_p[None, :]], 0), DS + 1, CP)

    shared = {
        "We1": np.ascontiguousarray(
            We1.reshape(8, 128, D).transpose(1, 0, 2)).astype(BF),   # [128,8,D]
        "We2": np.ascontiguousarray(
            We2.reshape(8, 128, D).transpose(1, 0, 2)).astype(BF),
        "We3b": We3b.astype(BF),
        "Wel1": np.ascontiguousarray(
            Wel1p.reshape(3, 128, DLP).transpose(1, 0, 2)).astype(BF),
        "Wel2": np.ascontiguousarray(
            Wel2p.reshape(3, 128, DLP).transpose(1, 0, 2)).astype(BF),
        "bel": belp[None, :].astype(BF),
        "Wnu1": Wnu1.reshape(8, 128, D).astype(BF),
        "Wnu2": Wnu2.reshape(8, 128, D).astype(BF),
        "bnu": b_nu.reshape(8, 128).T.astype(F32).copy(),
        "Wnul1": Wnul1p.reshape(3, 128, DLP).astype(BF),
        "Wnul2": Wnul2p.reshape(3, 128, DLP).astype(BF),
        "bnul": bnulp_layout(b_nul),
        "Wp1": Wp1_layout(Wp1),
        "Wp2": Wp2_layout(Wp2),
        "Wp4": Wp2_layout(Wp4),
        "Wp5": Wp1_layout(Wp5),
        "Wp3b": Wp3b.astype(BF),
        "ident8": np.eye(128, dtype=np.float32).astype(F8),
    }

    w2vp = np.zeros((N, DLP), F32)
    w2vp[:, :DL] = w2v_n

    in_maps = []
    for c in range(CORES):
        sl = slice(c * NS, (c + 1) * NS)
        srcg = np.stack([
            _gather_idx_layout(src_pad[c, g], GMAX // 16) for g in range(NGRP)
        ])
        ro_sl = slice(c * RO_LOC, (c + 1) * RO_LOC)
        sroT = np.concatenate(
            [s_f_ro[ro_sl].T, np.ones((1, RO_LOC), F32)], 0).astype(BF)

        m = dict(shared)
        m.update({
            "ftT": np.ascontiguousarray(feat_n[sl].T).astype(BF),
            "w2vT": np.ascontiguousarray(w2vp[sl].T).astype(BF),
            "srcg": srcg.astype(np.int16),
            "oh": oh_dev[c].astype(F8),
            "ohT": ohT[c].astype(BF),
            "sfT": sfT[c].astype(BF),
            "invc": invc[c],
            "rod": _gather_idx_layout(_pflat(old2new[ro_d[ro_sl]], 0),
                                      RO_LOC // 16),
            "ros": _gather_idx_layout(_pflat(old2new[ro_s[ro_sl]], 1),
                                      RO_LOC // 16),
            "sroT": sroT,
        })
        in_maps.append(m)

    return in_maps


def _pflat(v, t):
    """Row index into the concatenated [CORES, 2, NS, CP] P_full table."""
    cn, ln = v // NS, v % NS
    return (cn * 2 + t) * NS + ln


def Wp1_layout(w):
    out = np.zeros((8, 128, CP), F32)
    out[:, :, : w.shape[1]] = w.reshape(8, 128, -1)
    return out.astype(BF)


def Wp2_layout(w):
    out = np.zeros((3, 128, CP), F32)
    wp = np.zeros((DLP, w.shape[1]), F32)
    wp[: w.shape[0]] = w
    out[:, :, : w.shape[1]] = wp.reshape(3, 128, -1)
    return out.astype(BF)


def CP_pad(w):
    return CP


def bnulp_layout(b):
    out = np.zeros((DLP,), F32)
    out[: len(b)] = b
    return out.reshape(3, 128).T.astype(F32).copy()


# --------------------------------------------------------------------------
# device kernel builder
# --------------------------------------------------------------------------

def build_kernel(phase=4, reps=1):
    nc = bacc.Bacc("TRN2", target_bir_lowering=False)
    P = lambda n, s, d: nc.declare_dram_parameter(n, list(s), d, isOutput=False)

    ftT = P("ftT", [D, NS], dt.bfloat16)
    w2vT = P("w2vT", [DLP, NS], dt.bfloat16)
    We1 = P("We1", [128, D // 128, D], dt.bfloat16)
    We2 = P("We2", [128, D // 128, D], dt.bfloat16)
    We3b = P("We3b", [DS + 1, D], dt.bfloat16)
    Wel1 = P("Wel1", [128, DLP // 128, DLP], dt.bfloat16)
    Wel2 = P("Wel2", [128, DLP // 128, DLP], dt.bfloat16)
    bel = P("bel", [1, DLP], dt.bfloat16)
    Wnu1 = P("Wnu1", [D // 128, 128, D], dt.bfloat16)
    Wnu2 = P("Wnu2", [D // 128, 128, D], dt.bfloat16)
    bnu = P("bnu", [128, 8], dt.float32)
    Wnul1 = P("Wnul1", [DLP // 128, 128, DLP], dt.bfloat16)
    Wnul2 = P("Wnul2", [DLP // 128, 128, DLP], dt.bfloat16)
    bnul = P("bnul", [128, 3], dt.float32)
    Wp1 = P("Wp1", [D // 128, 128, CP], dt.bfloat16)
    Wp2 = P("Wp2", [DLP // 128, 128, CP], dt.bfloat16)
    Wp4 = P("Wp4", [DLP // 128, 128, CP], dt.bfloat16)
    Wp5 = P("Wp5", [D // 128, 128, CP], dt.bfloat16)
    Wp3b = P("Wp3b", [DS + 1, CP], dt.bfloat16)
    srcg = P("srcg", [NGRP, 128, GMAX // 16], dt.int16)
    oh = P("oh", [NGRP, 128, GMAX], dt.float8e4)
    ohT = P("ohT", [NGRP, 128, GMAX], dt.bfloat16)
    sfT = P("sfT", [NGRP, DS + 1, GMAX], dt.bfloat16)
    invc = P("invc", [128, NGRP * GS], dt.float32)
    rod = P("rod", [128, RO_LOC // 16], dt.int16)
    ros = P("ros", [128, RO_LOC // 16], dt.int16)
    sroT = P("sroT", [DS + 1, RO_LOC], dt.bfloat16)
    ident8 = P("ident8", [128, 128], dt.float8e4)

    out = nc.dram_tensor("out", [RO_LOC, CP], dt.float32, kind="ExternalOutput")

    HD = D // 2
    A_lo = nc.dram_tensor("A_lo", [NS, HD], dt.float8e4)
    A_hi = nc.dram_tensor("A_hi", [NS, HD], dt.float8e4)
    AL_sh = nc.dram_tensor("AL_sh", [NS, ALP], dt.float8e4)
    A_full_lo = nc.dram_tensor("A_full_lo", [N, HD], dt.float8e4,
                               addr_space="Shared")
    A_full_hi = nc.dram_tensor("A_full_hi", [N, HD], dt.float8e4,
                               addr_space="Shared")
    AL_full = nc.dram_tensor("AL_full", [N, ALP], dt.float8e4,
                             addr_space="Shared")
    P_sh = nc.dram_tensor("P_sh", [2, NS, CP], dt.bfloat16)
    P_full = nc.dram_tensor("P_full", [2 * N, CP], dt.bfloat16,
                            addr_space="Shared")

    RG = [list(range(CORES))]

    def AG(src, dst):
        if phase == 6:
            nc.sync.dma_start(dst.ap()[:NS, :], src.ap())
        else:
            nc.gpsimd.collective_compute(
                "AllGather", mybir.AluOpType.bypass, replica_groups=RG,
                ins=[src.ap().opt()], outs=[dst.ap().opt()])

    with tile.TileContext(nc) as tc:
        nc.gpsimd.load_library(library_config.mlp)

        for _rep in range(reps):
          with tc.tile_pool(name="pers", bufs=1) as pers:
            ftT_sb = pers.tile([128, D // 128, NS], dt.bfloat16)
            nc.sync.dma_start(ftT_sb[:], ftT.ap().rearrange("(k p) n -> p k n", p=128))
            w2vT_sb = pers.tile([128, DLP // 128, NS], dt.bfloat16)
            nc.sync.dma_start(w2vT_sb[:], w2vT.ap().rearrange("(k p) n -> p k n", p=128))
            We3b_sb = pers.tile([DS + 1, D], dt.bfloat16)
            nc.sync.dma_start(We3b_sb[:], We3b.ap())
            invc_sb = pers.tile([128, NGRP * GS], dt.float32)
            nc.sync.dma_start(invc_sb[:], invc.ap())
            id8_sb = pers.tile([128, 128], dt.float8e4)
            nc.sync.dma_start(id8_sb[:], ident8.ap())
            # agg in [Din, node] layout consumed by P3 (filled by xbar DMA)
            aggT_sb = pers.tile([128, NGRP, D // 128, 128], dt.bfloat16)
            agglT_sb = pers.tile([128, NGRP, DLP // 128, 128], dt.bfloat16)

            # --------------------------------------------------------------
            # P1: projections. A (fp8, two column halves) -> AG; AL -> AG;
            # B/BL stay in SBUF.
            # --------------------------------------------------------------
            from contextlib import ExitStack
            if True:
                _pbl = ExitStack()
                pbl = _pbl.enter_context(tc.tile_pool(name="pbl", bufs=1))
                B_sb = pbl.tile([128, NGRP, D], dt.bfloat16)
                BL_sb = pbl.tile([128, NGRP, DLP], dt.bfloat16)
                _p1 = ExitStack()
                pw = _p1.enter_context(tc.tile_pool(name="p1w", bufs=1))
                pio = _p1.enter_context(tc.tile_pool(name="p1io", bufs=3))
                pps = _p1.enter_context(
                    tc.tile_pool(name="p1ps", bufs=2, space="PSUM"))
                We1_sb = pw.tile([128, D // 128, D], dt.bfloat16)
                nc.sync.dma_start(We1_sb[:], We1.ap())
                We2_sb = pw.tile([128, D // 128, D], dt.bfloat16)
                nc.sync.dma_start(We2_sb[:], We2.ap())
                Wel1_sb = pw.tile([128, DLP // 128, DLP], dt.bfloat16)
                nc.sync.dma_start(Wel1_sb[:], Wel1.ap())
                Wel2_sb = pw.tile([128, DLP // 128, DLP], dt.bfloat16)
                nc.sync.dma_start(Wel2_sb[:], Wel2.ap())
                bel_sb = pw.tile([1, DLP], dt.bfloat16)
                nc.sync.dma_start(bel_sb[:], bel.ap())
                ones_sb = pw.tile([1, 128], dt.bfloat16)
                nc.vector.memset(ones_sb[:], 1.0)

                # A halves -> fp8 DRAM -> AG
                for half, A_t, A_f in ((0, A_lo, A_full_lo), (1, A_hi, A_full_hi)):
                    for mo in range(NS // 128):
                        msl = bass.ts(mo, 128)
                        psA = pps.tile([128, 512], dt.float32, space="PSUM",
                                       name="psA")
                        for k in range(D // 128):
                            nc.tensor.matmul(
                                psA[:], lhsT=ftT_sb[:, k, msl],
                                rhs=We1_sb[:, k, bass.ts(half, 512)],
                                start=(k == 0), stop=(k == D // 128 - 1))
                        stA = pio.tile([128, 512], dt.float8e4, name="stA")
                        nc.vector.tensor_copy(stA[:], psA[:])
                        nc.sync.dma_start(A_t[msl, :], stA[:])
                    AG(A_t, A_f)

                # B -> SBUF resident (gates P2's msg matmuls; do it early)
                for mo in range(NS // 128):
                    msl = bass.ts(mo, 128)
                    for nn in range(2):
                        psB = pps.tile([128, 512], dt.float32, space="PSUM",
                                       name="psB")
                        for k in range(D // 128):
                            nc.tensor.matmul(
                                psB[:], lhsT=ftT_sb[:, k, msl],
                                rhs=We2_sb[:, k, bass.ts(nn, 512)],
                                start=(k == 0), stop=(k == D // 128 - 1))
                        nc.vector.tensor_copy(B_sb[:, mo, bass.ts(nn, 512)],
                                              psB[:])

                # AL -> fp8 DRAM (rows padded to 512B) -> AG
                for mo in range(NS // 128):
                    msl = bass.ts(mo, 128)
                    psL = pps.tile([128, DLP], dt.float32, space="PSUM",
                                   name="psL")
                    for k in range(DLP // 128):
                        nc.tensor.matmul(
                            psL[:], lhsT=w2vT_sb[:, k, msl],
                            rhs=Wel1_sb[:, k, :],
                            start=(k == 0), stop=(k == DLP // 128 - 1))
                    stL = pio.tile([128, DLP], dt.float8e4, name="stL")
                    nc.vector.tensor_copy(stL[:], psL[:])
                    nc.sync.dma_start(AL_sh[msl, :DLP], stL[:])
                AG(AL_sh, AL_full)

                # BL -> SBUF resident
                for mo in range(NS // 128):
                    msl = bass.ts(mo, 128)
                    psBL = pps.tile([128, DLP], dt.float32, space="PSUM",
                                    name="psBL")
                    for k in range(DLP // 128):
                        nc.tensor.matmul(
                            psBL[:], lhsT=w2vT_sb[:, k, msl],
                            rhs=Wel2_sb[:, k, :],
                            start=(k == 0), stop=False)
                    nc.tensor.matmul(psBL[:], lhsT=ones_sb[:], rhs=bel_sb[:],
                                     start=False, stop=True)
                    nc.vector.tensor_copy(BL_sb[:, mo, :], psBL[:])

                _p1.close()

                # ----------------------------------------------------------
                # P2: edge phase, one 128-dst group at a time
                # ----------------------------------------------------------
                with (
                    tc.tile_pool(name="p2g", bufs=2) as p2g,
                    tc.tile_pool(name="p2m", bufs=2) as p2m,
                    tc.tile_pool(name="p2agg", bufs=2) as p2agg,
                    tc.tile_pool(name="p2ps", bufs=2, space="PSUM") as p2ps,
                    tc.tile_pool(name="p2psagg", bufs=1, space="PSUM") as p2psagg,
                ):
                    for g in range(NGRP):
                        src_i = p2g.tile([128, GMAX // 16], dt.int16,
                                         name="src_i")
                        nc.sync.dma_start(src_i[:], srcg.ap()[g])
                        oh_sb = p2g.tile([128, GS, 128], dt.float8e4,
                                         name="oh_sb")
                        nc.sync.dma_start(
                            oh_sb[:], oh.ap()[g].rearrange("p (s d) -> p s d", d=128))
                        ohT_sb = p2g.tile([128, GMAX], dt.bfloat16,
                                          name="ohT_sb")
                        nc.sync.dma_start(ohT_sb[:], ohT.ap()[g])
                        sfT_sb = p2g.tile([DS + 1, GMAX], dt.bfloat16,
                                          name="sfT_sb")
                        nc.sync.dma_start(sfT_sb[:], sfT.ap()[g])

                        a_lo = p2g.tile([128, GS, HD], dt.float8e4, name="a_lo")
                        nc.gpsimd.dma_gather(
                            out_ap=a_lo[:], in_ap=A_full_lo.ap(),
                            idxs_ap=src_i[:], num_idxs=GMAX,
                            num_idxs_reg=GMAX, elem_size=HD,
                            single_packet=False)
                        a_hi = p2g.tile([128, GS, HD], dt.float8e4, name="a_hi")
                        nc.gpsimd.dma_gather(
                            out_ap=a_hi[:], in_ap=A_full_hi.ap(),
                            idxs_ap=src_i[:], num_idxs=GMAX,
                            num_idxs_reg=GMAX, elem_size=HD,
                            single_packet=False)
                        al_g = p2g.tile([128, GS, ALP], dt.float8e4, name="al_g")
                        nc.gpsimd.dma_gather(
                            out_ap=al_g[:], in_ap=AL_full.ap(),
                            idxs_ap=src_i[:], num_idxs=GMAX,
                            num_idxs_reg=GMAX, elem_size=ALP,
                            single_packet=False)

                        m_t = p2m.tile([128, GS, D], dt.float8e4, name="m_t")
                        ml_t = p2m.tile([128, GS, DLP], dt.float8e4,
                                        name="ml_t", bufs=1)
                        for s in range(GS):
                            ssl = bass.ts(s, 128)
                            ps_m = p2ps.tile([128, D], dt.float32,
                                             space="PSUM", name="ps_m")
                            for h, a_h in ((0, a_lo), (1, a_hi)):
                                hsl = bass.ts(h, 512)
                                nc.tensor.matmul(
                                    ps_m[:, hsl], lhsT=sfT_sb[:, ssl],
                                    rhs=We3b_sb[:, hsl],
                                    start=True, stop=False)
                                nc.tensor.matmul(
                                    ps_m[:, hsl], lhsT=ohT_sb[:, ssl],
                                    rhs=B_sb[:, g, hsl],
                                    start=False, stop=True)
                                nc.vector.tensor_tensor(
                                    out=m_t[:, s, hsl], in0=a_h[:, s, :],
                                    in1=ps_m[:, hsl], op=mybir.AluOpType.add)
                            ps_ml = p2ps.tile([128, DLP], dt.float32,
                                              space="PSUM", name="ps_ml",
                                              bufs=1)
                            nc.tensor.matmul(ps_ml[:], lhsT=ohT_sb[:, ssl],
                                             rhs=BL_sb[:, g, :],
                                             start=True, stop=True)
                            nc.vector.tensor_tensor(
                                out=ml_t[:, s, :], in0=al_g[:, s, :DLP],
                                in1=ps_ml[:], op=mybir.AluOpType.add)
                            sc = invc_sb[:, g * GS + s : g * GS + s + 1]
                            nc.scalar.activation(
                                m_t[:, s, :], m_t[:, s, :],
                                mybir.ActivationFunctionType.Relu, scale=sc)
                            nc.scalar.activation(
                                ml_t[:, s, :], ml_t[:, s, :],
                                mybir.ActivationFunctionType.Relu, scale=sc)

                        # aggregation into [dst, D] layout
                        ps_ag = p2psagg.tile([128, D + DLP], dt.float32,
                                             space="PSUM", name="ps_ag")
                        if DOUBLE_ROW:
                            for sp in range(GS // 2):
                                s2 = slice(2 * sp, 2 * sp + 2)
                                last = sp == GS // 2 - 1
                                for h in range(2):
                                    hsl = bass.ts(h, 512)
                                    nc.tensor.matmul(
                                        ps_ag[:, hsl], lhsT=oh_sb[:, s2, :],
                                        rhs=m_t[:, s2, hsl],
                                        start=(sp == 0), stop=last,
                                        perf_mode=mybir.MatmulPerfMode.DoubleRow)
                                nc.tensor.matmul(
                                    ps_ag[:, D : D + DLP],
                                    lhsT=oh_sb[:, s2, :],
                                    rhs=ml_t[:, s2, :],
                                    start=(sp == 0), stop=last,
                                    perf_mode=mybir.MatmulPerfMode.DoubleRow)
                        else:
                            for s in range(GS):
                                last = s == GS - 1
                                for h in range(2):
                                    hsl = bass.ts(h, 512)
                                    nc.tensor.matmul(
                                        ps_ag[:, hsl], lhsT=oh_sb[:, s, :],
                                        rhs=m_t[:, s, hsl],
                                        start=(s == 0), stop=last)
                                nc.tensor.matmul(
                                    ps_ag[:, D : D + DLP], lhsT=oh_sb[:, s, :],
                                    rhs=ml_t[:, s, :],
                                    start=(s == 0), stop=last)

                        agg_d = p2agg.tile([128, D + DLP], dt.bfloat16,
                                           name="agg_d")
                        nc.vector.tensor_copy(agg_d[:], ps_ag[:])
                        # xbar transpose [dst, D] -> [Din, dst]
                        nc.sync.dma_start(
                            aggT_sb[:, g, :, :], agg_d[:, :D], transpose=True)
                        nc.sync.dma_start(
                            agglT_sb[:, g, :, :], agg_d[:, D:], transpose=True)

                _pbl.close()

                # ----------------------------------------------------------
                # P3: node update + predictor projections (transposed)
                # ----------------------------------------------------------
                with (
                    tc.tile_pool(name="p3w", bufs=1) as p3w,
                    tc.tile_pool(name="p3n", bufs=2) as p3n,
                    tc.tile_pool(name="p3ps", bufs=2, space="PSUM") as p3ps,
                    tc.tile_pool(name="p3ps2", bufs=2, space="PSUM") as p3ps2,
                ):
                    w1all = p3w.tile([128, D // 128, D], dt.bfloat16)
                    nc.sync.dma_start(w1all[:], Wnu1.ap().rearrange("k p d -> p k d"))
                    w2all = p3w.tile([128, D // 128, D], dt.bfloat16)
                    nc.sync.dma_start(w2all[:], Wnu2.ap().rearrange("k p d -> p k d"))
                    wl1all = p3w.tile([128, DLP // 128, DLP], dt.bfloat16)
                    nc.sync.dma_start(wl1all[:], Wnul1.ap().rearrange("k p d -> p k d"))
                    wl2all = p3w.tile([128, DLP // 128, DLP], dt.bfloat16)
                    nc.sync.dma_start(wl2all[:], Wnul2.ap().rearrange("k p d -> p k d"))
                    bnu_sb = p3w.tile([128, 8], dt.float32)
                    nc.sync.dma_start(bnu_sb[:], bnu.ap())
                    bnul_sb = p3w.tile([128, 3], dt.float32)
                    nc.sync.dma_start(bnul_sb[:], bnul.ap())
                    wp1_sb = p3w.tile([128, D // 128, CP], dt.bfloat16)
                    nc.sync.dma_start(wp1_sb[:], Wp1.ap().rearrange("k p c -> p k c"))
                    wp2_sb = p3w.tile([128, DLP // 128, CP], dt.bfloat16)
                    nc.sync.dma_start(wp2_sb[:], Wp2.ap().rearrange("k p c -> p k c"))
                    wp4_sb = p3w.tile([128, DLP // 128, CP], dt.bfloat16)
                    nc.sync.dma_start(wp4_sb[:], Wp4.ap().rearrange("k p c -> p k c"))
                    wp5_sb = p3w.tile([128, D // 128, CP], dt.bfloat16)
                    nc.sync.dma_start(wp5_sb[:], Wp5.ap().rearrange("k p c -> p k c"))
                    PdT_sb = p3w.tile([128, NS], dt.bfloat16)
                    PsT_sb = p3w.tile([128, NS], dt.bfloat16)

                    for n in range(NS // 512):
                        nsl = bass.ts(n, 512)
                        nf_n = p3n.tile([128, D // 128, 512], dt.bfloat16,
                                        name="nf_n")
                        nfl_n = p3n.tile([128, DLP // 128, 512], dt.bfloat16,
                                         name="nfl_n")
                        for mo in range(D // 128):
                            psN = p3ps.tile([128, 512], dt.float32,
                                            space="PSUM", name="psN")
                            for k in range(D // 128):
                                nc.tensor.matmul(
                                    psN[:], lhsT=w1all[:, k, bass.ts(mo, 128)],
                                    rhs=ftT_sb[:, k, nsl],
                                    start=(k == 0), stop=False)
                            for k in range(D // 128):
                                nc.tensor.matmul(
                                    psN[:], lhsT=w2all[:, k, bass.ts(mo, 128)],
                                    rhs=aggT_sb[:, 4 * n : 4 * n + 4, k, :],
                                    start=False, stop=(k == D // 128 - 1))
                            nc.scalar.activation(
                                nf_n[:, mo, :], psN[:],
                                mybir.ActivationFunctionType.Relu,
                                bias=bnu_sb[:, mo : mo + 1])
                        for mo in range(DLP // 128):
                            psL2 = p3ps.tile([128, 512], dt.float32,
                                             space="PSUM", name="psL2")
                            for k in range(DLP // 128):
                                nc.tensor.matmul(
                                    psL2[:], lhsT=wl1all[:, k, bass.ts(mo, 128)],
                                    rhs=w2vT_sb[:, k, nsl],
                                    start=(k == 0), stop=False)
                            for k in range(DLP // 128):
                                nc.tensor.matmul(
                                    psL2[:], lhsT=wl2all[:, k, bass.ts(mo, 128)],
                                    rhs=agglT_sb[:, 4 * n : 4 * n + 4, k, :],
                                    start=False, stop=(k == DLP // 128 - 1))
                            nc.scalar.activation(
                                nfl_n[:, mo, :], psL2[:],
                                mybir.ActivationFunctionType.Relu,
                                bias=bnul_sb[:, mo : mo + 1])
                        # predictor projections, [CP, node] orientation
                        for PT_sb, wa, wb in ((PdT_sb, wp1_sb, wp2_sb),
                                              (PsT_sb, wp5_sb, wp4_sb)):
                            psP = p3ps2.tile([128, 512], dt.float32,
                                             space="PSUM", name="psP")
                            for k in range(D // 128):
                                nc.tensor.matmul(
                                    psP[:], lhsT=wa[:, k, :],
                                    rhs=nf_n[:, k, :],
                                    start=(k == 0), stop=False)
                            for k in range(DLP // 128):
                                nc.tensor.matmul(
                                    psP[:], lhsT=wb[:, k, :],
                                    rhs=nfl_n[:, k, :],
                                    start=False, stop=(k == DLP // 128 - 1))
                            nc.vector.tensor_copy(PT_sb[:, nsl], psP[:])

                    # transpose [CP, node] -> rows, store, one all-gather
                    for t, PT_sb in ((0, PdT_sb), (1, PsT_sb)):
                        P_rows = p3n.tile([128, NS // 128, 128], dt.bfloat16,
                                          name="P_rows")
                        nc.sync.dma_start(P_rows[:], PT_sb[:], transpose=True)
                        nc.sync.dma_start(
                            P_sh.ap()[t].rearrange("(b p) c -> p b c", p=128),
                            P_rows[:])
                    if phase == 6:
                        nc.sync.dma_start(
                            P_full.ap()[: 2 * NS, :],
                            P_sh.ap().rearrange("t n c -> (t n) c"))
                    else:
                        nc.gpsimd.collective_compute(
                            "AllGather", mybir.AluOpType.bypass,
                            replica_groups=RG,
                            ins=[P_sh.ap().opt()], outs=[P_full.ap().opt()])

            # --------------------------------------------------------------
            # P4: readout
            # --------------------------------------------------------------
            with (
                tc.tile_pool(name="p4", bufs=1) as p4,
                tc.tile_pool(name="p4ps", bufs=2, space="PSUM") as p4ps,
            ):
                rod_sb = p4.tile([128, RO_LOC // 16], dt.int16)
                nc.sync.dma_start(rod_sb[:], rod.ap())
                ros_sb = p4.tile([128, RO_LOC // 16], dt.int16)
                nc.sync.dma_start(ros_sb[:], ros.ap())
                sroT_sb = p4.tile([DS + 1, RO_LOC], dt.bfloat16)
                nc.sync.dma_start(sroT_sb[:], sroT.ap())
                wp3_sb = p4.tile([DS + 1, CP], dt.bfloat16)
                nc.sync.dma_start(wp3_sb[:], Wp3b.ap())

                pd_g = p4.tile([128, ROS, CP], dt.bfloat16)
                nc.gpsimd.dma_gather(
                    out_ap=pd_g[:], in_ap=P_full.ap(), idxs_ap=rod_sb[:],
                    num_idxs=RO_LOC, num_idxs_reg=RO_LOC, elem_size=CP,
                    single_packet=False)
                ps_g = p4.tile([128, ROS, CP], dt.bfloat16)
                nc.gpsimd.dma_gather(
                    out_ap=ps_g[:], in_ap=P_full.ap(), idxs_ap=ros_sb[:],
                    num_idxs=RO_LOC, num_idxs_reg=RO_LOC, elem_size=CP,
                    single_packet=False)

                pred = p4.tile([128, ROS, CP], dt.float32)
                nc.vector.tensor_tensor(
                    out=pred[:], in0=pd_g[:], in1=ps_g[:],
                    op=mybir.AluOpType.add)
                for s in range(ROS):
                    psR = p4ps.tile([128, CP], dt.float32, space="PSUM",
                                    name="psR")
                    nc.tensor.matmul(psR[:], lhsT=sroT_sb[:, bass.ts(s, 128)],
                                     rhs=wp3_sb[:], start=True, stop=True)
                    nc.vector.tensor_tensor(
                        out=pred[:, s, :], in0=pred[:, s, :],
                        in1=psR[:], op=mybir.AluOpType.add)
                nc.sync.dma_start(
                    out.ap().rearrange("(s p) c -> p s c", p=128), pred[:])

    nc.compile()
    return nc


# --------------------------------------------------------------------------
# entry point
# --------------------------------------------------------------------------

_CACHE = {}


def _get_kernel():
    phase = int(os.environ.get("KPHASE", "4"))
    reps = int(os.environ.get("KREPS", "1"))
    key = (phase, reps)
    if key not in _CACHE:
        _CACHE[key] = build_kernel(phase, reps)
    return _CACHE[key]


def _run(inputs, trace=False):
    in_maps = _preprocess(inputs)
    nc = _get_kernel()
    res = run_bass_kernel_spmd(nc, in_maps, core_ids=list(range(CORES)),
                               trace=trace)
    pred = np.concatenate([np.asarray(res.results[c]["out"])
                           for c in range(CORES)], 0)[:, :C]
    return np.ascontiguousarray(pred.astype(np.float32)), res


def kernel(**inputs):
    out, _ = _run(inputs, trace=False)
    return out


def kernel_with_profile(inputs):
    return _run(inputs, trace=False)
